# revision 15
# baseline (speedup 1.0000x reference)
"""ConvAttention Trainium2 kernel.

Strategy (8 NeuronCores, zero collectives):
  - Fold depthwise-conv + BN + pointwise-conv into 3 shift matrices per set:
      q_t = sum_j Wq_j @ x_{t+j-1} + beta_q   (same for k, v)
    (host-side numpy on the tiny weight tensors; Q-side pre-scaled by 1/sqrt(DK))
  - Shard by (batch, T/4): core i handles batch i//4, queries [(i%4)*1024, +1024),
    all 8 heads.  Each core computes K/V over the full sequence of its batch
    (redundant 4x, but cheap) and writes a disjoint [1024, 256] output slice.
  - On device: S^T = K @ Q^T per 128-key block (keys on partitions), exp on the
    scalar engine (scores are in [-2.5, 2.5] for this model family => no
    max-subtraction needed; exp is mathematically shift-invariant in softmax),
    context accumulated via [V | 1] augmented matmul (denominator comes out as
    row 32), normalize via 1/denom broadcast matmul, out-projection per head
    accumulated in PSUM, + out_b, DMA out.
"""

import os
import threading

import numpy as np
import ml_dtypes

B, T, D, H, KW = 2, 4096, 256, 8, 3
DK = D // H  # 32
EPS = 1e-5
NCORES = 8
QS = T // 4  # 1024 queries per core
TPAD = 4128  # T+2 padded up (mult of 16 for dma transpose rows)
QPAD = 1056  # QS+2 padded up
CA = DK + 1  # 33 = V columns per head incl. ones column
QB = 512  # query block (psum free dim)
NKB = T // 128  # 32 key blocks

_lock = threading.Lock()
_cached = {}


def _fold_weights(dw_w, dw_b, bn_gamma, bn_beta, bn_mean, bn_var, pw_w, pw_b):
    """Returns Wj [3set, 3j, D(out), D(in)] f32 and beta [3set, D] f32."""
    Wj = np.zeros((3, KW, D, D), dtype=np.float64)
    beta = np.zeros((3, D), dtype=np.float64)
    for s in range(3):
        sc = bn_gamma[s] / np.sqrt(bn_var[s] + EPS)
        wprime = dw_w[s, :, 0, :] * sc[:, None]  # [c, j]
        bprime = (dw_b[s] - bn_mean[s]) * sc + bn_beta[s]
        for j in range(KW):
            Wj[s, j] = pw_w[s] * wprime[None, :, j]  # [o, c]
        beta[s] = pw_w[s] @ bprime + pw_b[s]
    # fold the 1/sqrt(DK) score scale into the Q projection
    Wj[0] /= np.sqrt(DK)
    beta[0] /= np.sqrt(DK)
    return Wj.astype(np.float32), beta.astype(np.float32)


def _build_nc(debug_dump=False, reps=1):
    import concourse.bacc as bacc
    import concourse.bass as bass
    import concourse.mybir as mybir
    import concourse.tile as tile

    f32 = mybir.dt.float32
    bf16 = mybir.dt.bfloat16
    AF = mybir.ActivationFunctionType
    ALU = mybir.AluOpType

    nc = bacc.Bacc("TRN2", target_bir_lowering=False, debug=False,
                   num_devices=NCORES, enable_partition_id=False)

    xb_d = nc.dram_tensor("xb", [TPAD, D], bf16, kind="ExternalInput")
    xq_d = nc.dram_tensor("xq", [QPAD, D], bf16, kind="ExternalInput")
    # [c(128), set(2:q,k), j(3), cb(2), db(2), d(128)]
    wqk_d = nc.dram_tensor("wqk", [128, 2, KW, 2, 2, 128], bf16,
                           kind="ExternalInput")
    # [c(128), j(3), cb(2), col(264)]
    wv_d = nc.dram_tensor("wv", [128, KW, 2, H * CA], bf16, kind="ExternalInput")
    # [d(128), set(2), db(2)]
    bqk_d = nc.dram_tensor("bqk", [128, 2, 2], f32, kind="ExternalInput")
    vb_d = nc.dram_tensor("vbias", [H * CA], f32, kind="ExternalInput")
    # [dd(32), h(8), o(256)]
    outw_d = nc.dram_tensor("outw", [DK, H, D], bf16, kind="ExternalInput")
    outb_d = nc.dram_tensor("outb", [D], f32, kind="ExternalInput")
    out_d = nc.dram_tensor("out", [QS, D], f32, kind="ExternalOutput")
    dbg = {}
    if debug_dump:
        dbg["kt"] = nc.dram_tensor("dbg_kt", [128, 2, T], bf16,
                                   kind="ExternalOutput")
        dbg["qt"] = nc.dram_tensor("dbg_qt", [128, 2, QS], bf16,
                                   kind="ExternalOutput")
        dbg["vt"] = nc.dram_tensor("dbg_vt", [128, NKB, H * CA], bf16,
                                   kind="ExternalOutput")
        dbg["ct"] = nc.dram_tensor("dbg_ct", [2, DK, H, QB], bf16,
                                   kind="ExternalOutput")
        dbg["pt"] = nc.dram_tensor("dbg_pt", [128, 3, QB], bf16,
                                   kind="ExternalOutput")
        dbg["nrm"] = nc.dram_tensor("dbg_nrm", [4, CA, QB], f32,
                                    kind="ExternalOutput")

    with tile.TileContext(nc) as tc:
        with (
            tc.tile_pool(name="consts", bufs=1) as consts,
            tc.tile_pool(name="work", bufs=3) as work,
            tc.tile_pool(name="psum", bufs=2, space=bass.MemorySpace.PSUM) as psum,
        ):
            # ---- constants / weights ----
            wqk_sb = consts.tile([128, 2, KW, 2, 2, 128], bf16)
            nc.sync.dma_start(wqk_sb[:], wqk_d[:])
            wv_sb = consts.tile([128, KW, 2, H * CA], bf16)
            nc.sync.dma_start(wv_sb[:], wv_d[:])
            bqk_sb = consts.tile([128, 2, 2], f32)
            nc.sync.dma_start(bqk_sb[:], bqk_d[:])
            outw_sb = consts.tile([DK, H, D], bf16)
            nc.sync.dma_start(outw_sb[:], outw_d[:])
            # partition-broadcast bias rows
            vb_sb = consts.tile([128, H * CA], f32)
            vb_ap = bass.AP(tensor=vb_d, offset=0, ap=[[0, 128], [1, H * CA]])
            nc.sync.dma_start(vb_sb[:], vb_ap)
            outb_sb = consts.tile([128, D], f32)
            outb_ap = bass.AP(tensor=outb_d, offset=0, ap=[[0, 128], [1, D]])
            nc.sync.dma_start(outb_sb[:], outb_ap)
            ones_sb = consts.tile([CA, DK], f32)
            nc.vector.memset(ones_sb[:], 1.0)

            # ---- x^T loads (DMA transpose, bf16) ----
            xbT = consts.tile([128, 2, TPAD], bf16)
            xqT = consts.tile([128, 2, QPAD], bf16)
            for cb in range(2):
                nc.sync.dma_start_transpose(
                    xbT[:, cb, :], xb_d[:, cb * 128:(cb + 1) * 128])
                nc.sync.dma_start_transpose(
                    xqT[:, cb, :], xq_d[:, cb * 128:(cb + 1) * 128])

            KT = consts.tile([128, 2, T], bf16)
            QT = consts.tile([128, 2, QS], bf16)
            Vt = consts.tile([128, NKB, H * CA], bf16)

            def proj_qk():
                for dst, src, n_t, s in ((KT, xbT, T, 1), (QT, xqT, QS, 0)):
                    for db in range(2):
                        for tt in range(n_t // QB):
                            ps = psum.tile([128, 3, QB], f32, tag="scores",
                                           name="ps_proj")
                            first = True
                            for j in range(KW):
                                for cb in range(2):
                                    nc.tensor.matmul(
                                        ps[:, 0, :],
                                        lhsT=wqk_sb[:, s, j, cb, db, :],
                                        rhs=src[:, cb,
                                                tt * QB + j: tt * QB + j + QB],
                                        start=first,
                                        stop=(j == KW - 1 and cb == 1))
                                    first = False
                            nc.vector.tensor_scalar_add(
                                dst[:, db, tt * QB:(tt + 1) * QB],
                                ps[:, 0, :], bqk_sb[:, s, db:db + 1])

            def proj_v():
                for kb in range(NKB):
                    ps = psum.tile([128, 3, QB], f32, tag="scores", name="ps_v")
                    first = True
                    for j in range(KW):
                        for cb in range(2):
                            nc.tensor.matmul(
                                ps[:, 0, :H * CA],
                                lhsT=xbT[:, cb, kb * 128 + j: kb * 128 + j + 128],
                                rhs=wv_sb[:, j, cb, :],
                                start=first, stop=(j == KW - 1 and cb == 1))
                            first = False
                    nc.vector.tensor_add(Vt[:, kb, :], ps[:, 0, :H * CA],
                                         vb_sb[:])

            # key-block groups of 3 (psum: 2x3 score banks + 2x1 ctx banks = 8)
            groups = [(k, min(k + 3, NKB)) for k in range(0, NKB, 3)]

            def attention(dump=False):
                for qb in range(QS // QB):
                    CT = work.tile([DK, H, QB], bf16, tag="CT", bufs=2)
                    for h in range(H):
                        hp, hq = h % 4, h // 4
                        ctxp = psum.tile([CA, QB], f32, tag="ctx", name="ctxp")
                        for (k0, k1) in groups:
                            nkb = k1 - k0
                            sp = psum.tile([128, 3, QB], f32, tag="scores",
                                           name="sp")
                            for i in range(nkb):
                                kb = k0 + i
                                nc.tensor.matmul(
                                    sp[:, i, :],
                                    lhsT=KT[hp * 32:(hp + 1) * 32, hq,
                                            kb * 128:(kb + 1) * 128],
                                    rhs=QT[hp * 32:(hp + 1) * 32, hq,
                                           qb * QB:(qb + 1) * QB],
                                    start=True, stop=True,
                                    tile_position=(hp * 32, 0))
                            PT = work.tile([128, 3, QB], bf16, tag="PT")
                            nc.scalar.activation(PT[:, :nkb, :], sp[:, :nkb, :],
                                                 AF.Exp)
                            if dump and qb == 0 and h == 0 and k0 == 0:
                                nc.sync.dma_start(dbg["pt"][:], PT[:])
                            for i in range(nkb):
                                kb = k0 + i
                                nc.tensor.matmul(
                                    ctxp[:],
                                    lhsT=Vt[:, kb, h * CA:(h + 1) * CA],
                                    rhs=PT[:, i, :],
                                    start=(kb == 0), stop=(kb == NKB - 1))
                        # normalize: CT = ctx_unnorm * bcast(1/denom)
                        den_sb = work.tile([CA, QB], f32, tag="den")
                        nc.vector.tensor_copy(den_sb[DK:CA, :], ctxp[DK:CA, :])
                        rec_sb = work.tile([CA, QB], f32, tag="rec")
                        nc.vector.reciprocal(rec_sb[DK:CA, :], den_sb[DK:CA, :])
                        bc = psum.tile([DK, QB], f32, tag="scores", name="bc")
                        nc.tensor.matmul(bc[:], lhsT=ones_sb[DK:CA, :],
                                         rhs=rec_sb[DK:CA, :],
                                         start=True, stop=True,
                                         tile_position=(32, 0))
                        bc_sb = work.tile([DK, QB], f32, tag="bcsb")
                        nc.vector.tensor_copy(bc_sb[:], bc[:])
                        if dump and qb == 0 and h == 0:
                            ctxu_sb = work.tile([CA, QB], f32, tag="dbgc")
                            nc.vector.tensor_copy(ctxu_sb[:], ctxp[:])
                            nc.sync.dma_start(dbg["nrm"][0], ctxu_sb[:])
                            nc.sync.dma_start(dbg["nrm"][1, :, :], den_sb[:])
                            nc.sync.dma_start(dbg["nrm"][2, :, :], rec_sb[:])
                            nc.sync.dma_start(dbg["nrm"][3, :DK, :], bc_sb[:])
                        nc.vector.tensor_tensor(
                            CT[:, h, :], ctxp[:DK, :], bc_sb[:], ALU.mult)
                    if dump:
                        nc.sync.dma_start(dbg["ct"][qb], CT[:])
                    # out-projection for this q block
                    for qs in range(QB // 128):
                        op = psum.tile([128, 3, QB], f32, tag="scores", name="op")
                        for h in range(H):
                            nc.tensor.matmul(
                                op[:, 0, :D],
                                lhsT=CT[:, h, qs * 128:(qs + 1) * 128],
                                rhs=outw_sb[:, h, :],
                                start=(h == 0), stop=(h == H - 1))
                        osb = work.tile([128, D], f32, tag="osb")
                        nc.vector.tensor_add(osb[:], op[:, 0, :D], outb_sb[:])
                        nc.sync.dma_start(
                            out_d[qb * QB + qs * 128: qb * QB + (qs + 1) * 128, :],
                            osb[:])

            for rep in range(reps):
                proj_qk()
                proj_v()
                if debug_dump and rep == 0:
                    nc.sync.dma_start(dbg["kt"][:], KT[:])
                    nc.sync.dma_start(dbg["qt"][:], QT[:])
                    nc.sync.dma_start(dbg["vt"][:], Vt[:])
                attention(dump=debug_dump and rep == 0)

    nc.compile()
    return nc


def _prep_inputs(x, dw_w, dw_b, bn_gamma, bn_beta, bn_mean, bn_var,
                 pw_w, pw_b, out_w, out_b):
    """Host-side arrangement of per-core input dicts."""
    bf = ml_dtypes.bfloat16
    Wj, beta = _fold_weights(dw_w, dw_b, bn_gamma, bn_beta, bn_mean,
                             bn_var, pw_w, pw_b)

    # wqk [c, set, j, cb, db, d] = Wj[set, j, db*128+d, cb*128+c]
    w2 = Wj[:2].reshape(2, KW, 2, 128, 2, 128)  # [set, j, db, d, cb, c]
    wqk = np.ascontiguousarray(w2.transpose(5, 0, 1, 4, 2, 3)).astype(bf)

    # wv [c, j, cb, col] with col = h*33+dd (dd<32), ones col zero-weight
    wv = np.zeros((128, KW, 2, H * CA), dtype=np.float32)
    wv3 = Wj[2].reshape(KW, H, DK, 2, 128)  # [j, h, dd, cb, c]
    for h in range(H):
        wv[:, :, :, h * CA:h * CA + DK] = wv3[:, h].transpose(3, 0, 2, 1)
    wv = wv.astype(bf)

    bqk = np.ascontiguousarray(
        beta[:2].reshape(2, 2, 128).transpose(2, 0, 1)).astype(np.float32)

    vbias = np.zeros((H * CA,), dtype=np.float32)
    vb3 = beta[2].reshape(H, DK)
    for h in range(H):
        vbias[h * CA:h * CA + DK] = vb3[h]
        vbias[h * CA + DK] = 1.0

    # outw [dd, h, o] = out_w[o, h*32+dd]
    outw = np.ascontiguousarray(
        out_w.reshape(D, H, DK).transpose(2, 1, 0)).astype(bf)
    outb = out_b.astype(np.float32)

    shared = dict(wqk=wqk, wv=wv, bqk=bqk, vbias=vbias, outw=outw, outb=outb)

    xpad = np.zeros((B, TPAD, D), dtype=bf)
    xpad[:, 1:T + 1, :] = x.astype(bf)

    in_maps = []
    for core in range(NCORES):
        b, q0 = core // 4, (core % 4) * QS
        m = dict(shared)
        m["xb"] = np.ascontiguousarray(xpad[b])
        m["xq"] = np.ascontiguousarray(xpad[b, q0:q0 + QPAD])
        in_maps.append(m)
    return in_maps


LAST_RESULTS = None


def _get_exec(reps=1):
    """Build the bass module once and wrap it in a cached, jitted 8-core
    shard_map callable (PJRT / axon path)."""
    key = ("exec", reps)
    if key in _cached:
        return _cached[key]
    import jax
    from jax.sharding import Mesh, PartitionSpec
    from jax.experimental.shard_map import shard_map
    import concourse.mybir as mybir
    from concourse import bass2jax

    bass2jax.install_neuronx_cc_hook()
    nc = _build_nc(reps=reps)

    in_names, out_names, out_avals = [], [], []
    for alloc in nc.m.functions[0].allocations:
        if not isinstance(alloc, mybir.MemoryLocationSet):
            continue
        name = alloc.memorylocations[0].name
        if alloc.kind == "ExternalInput":
            in_names.append(name)
        elif alloc.kind == "ExternalOutput":
            out_names.append(name)
            out_avals.append(jax.core.ShapedArray(
                tuple(alloc.tensor_shape), mybir.dt.np(alloc.dtype)))
    all_in_names = in_names + out_names  # outputs passed as zero inputs

    def _body(*args):
        outs = bass2jax._bass_exec_p.bind(
            *args,
            out_avals=tuple(out_avals),
            in_names=tuple(all_in_names),
            out_names=tuple(out_names),
            lowering_input_output_aliases=(),
            sim_require_finite=True,
            sim_require_nnan=True,
            nc=nc,
        )
        return tuple(outs)

    devices = jax.devices()[:NCORES]
    mesh = Mesh(np.asarray(devices), ("core",))
    n_all = len(in_names) + len(out_names)
    fn = jax.jit(
        shard_map(_body, mesh=mesh,
                  in_specs=(PartitionSpec("core"),) * n_all,
                  out_specs=(PartitionSpec("core"),) * len(out_names),
                  check_rep=False),
        keep_unused=True,
    )
    _cached[key] = (fn, in_names, out_names, out_avals, mesh)
    return _cached[key]


def _device_args(in_maps, reps=1):
    """Concat per-core inputs on axis 0 and device_put with core sharding."""
    import jax
    from jax.sharding import NamedSharding, PartitionSpec
    fn, in_names, out_names, out_avals, mesh = _get_exec(reps)
    sh = NamedSharding(mesh, PartitionSpec("core"))
    args = []
    for name in in_names:
        cat = np.concatenate([in_maps[c][name][None] for c in range(NCORES)],
                             axis=0)
        cat = cat.reshape(NCORES * cat.shape[1], *cat.shape[2:])
        args.append(jax.device_put(cat, sh))
    for av in out_avals:
        z = np.zeros((NCORES * av.shape[0], *av.shape[1:]), av.dtype)
        args.append(jax.device_put(z, sh))
    return args


def _run(args, reps=1):
    fn, in_names, out_names, out_avals, mesh = _get_exec(reps)
    outs = fn(*args)
    res = []
    for c in range(NCORES):
        res.append({name: np.asarray(outs[i]).reshape(
            NCORES, *out_avals[i].shape)[c] for i, name in enumerate(out_names)})
    return res, outs


def kernel(x, dw_w, dw_b, bn_gamma, bn_beta, bn_mean, bn_var,
           pw_w, pw_b, out_w, out_b):
    global LAST_RESULTS
    args = [np.asarray(a) for a in (x, dw_w, dw_b, bn_gamma, bn_beta, bn_mean,
                                    bn_var, pw_w, pw_b, out_w, out_b)]
    with _lock:
        in_maps = _prep_inputs(*args)
        dev_args = _device_args(in_maps)
        _cached["bench_args"] = dev_args
        results, _ = _run(dev_args)
    LAST_RESULTS = results

    out = np.empty((B, T, D), dtype=np.float32)
    for core in range(NCORES):
        b, q0 = core // 4, (core % 4) * QS
        out[b, q0:q0 + QS] = results[core]["out"]
    return out


def bench(n=6, reps=1):
    """Steady-state wall time of the jitted 8-core execution."""
    import time
    import jax
    fn, in_names, out_names, out_avals, mesh = _get_exec(reps)
    dev_args = _cached["bench_args"]
    # warmup for this reps variant
    outs = fn(*dev_args)
    jax.block_until_ready(outs)
    times = []
    for _ in range(n):
        t0 = time.perf_counter()
        outs = fn(*dev_args)
        jax.block_until_ready(outs)
        times.append(time.perf_counter() - t0)
    return times


# revision 20
# speedup vs baseline: 1.4388x; 1.4388x over previous
"""ConvAttention Trainium2 kernel.

Strategy (8 NeuronCores, zero collectives):
  - Fold depthwise-conv + BN + pointwise-conv into 3 shift matrices per set:
      q_t = sum_j Wq_j @ x_{t+j-1} + beta_q   (same for k, v)
    (host-side numpy on the tiny weight tensors; Q-side pre-scaled by 1/sqrt(DK))
  - Shard by (batch, T/4): core i handles batch i//4, queries [(i%4)*1024, +1024),
    all 8 heads.  Each core computes K/V over the full sequence of its batch
    (redundant 4x, but cheap) and writes a disjoint [1024, 256] output slice.
  - On device: S^T = K @ Q^T per 128-key block (keys on partitions), exp on the
    scalar engine (scores are in [-2.5, 2.5] for this model family => no
    max-subtraction needed; exp is mathematically shift-invariant in softmax),
    context accumulated via [V | 1] augmented matmul (denominator comes out as
    row 32), normalize via 1/denom broadcast matmul, out-projection per head
    accumulated in PSUM, + out_b, DMA out.
"""

import os
import threading

import numpy as np
import ml_dtypes

B, T, D, H, KW = 2, 4096, 256, 8, 3
DK = D // H  # 32
EPS = 1e-5
NCORES = 8
QS = T // 4  # 1024 queries per core
TPAD = 4128  # T+2 padded up (mult of 16 for dma transpose rows)
QPAD = 1056  # QS+2 padded up
CA = DK + 1  # 33 = V columns per head incl. ones column
QB = 512  # query block (psum free dim)
NKB = T // 128  # 32 key blocks

_lock = threading.Lock()
_cached = {}


def _fold_weights(dw_w, dw_b, bn_gamma, bn_beta, bn_mean, bn_var, pw_w, pw_b):
    """Returns Wj [3set, 3j, D(out), D(in)] f32 and beta [3set, D] f32."""
    Wj = np.zeros((3, KW, D, D), dtype=np.float64)
    beta = np.zeros((3, D), dtype=np.float64)
    for s in range(3):
        sc = bn_gamma[s] / np.sqrt(bn_var[s] + EPS)
        wprime = dw_w[s, :, 0, :] * sc[:, None]  # [c, j]
        bprime = (dw_b[s] - bn_mean[s]) * sc + bn_beta[s]
        for j in range(KW):
            Wj[s, j] = pw_w[s] * wprime[None, :, j]  # [o, c]
        beta[s] = pw_w[s] @ bprime + pw_b[s]
    # fold the 1/sqrt(DK) score scale into the Q projection
    Wj[0] /= np.sqrt(DK)
    beta[0] /= np.sqrt(DK)
    return Wj.astype(np.float32), beta.astype(np.float32)


def _build_nc(debug_dump=False, reps=1):
    import concourse.bacc as bacc
    import concourse.bass as bass
    import concourse.mybir as mybir
    import concourse.tile as tile

    f32 = mybir.dt.float32
    bf16 = mybir.dt.bfloat16
    AF = mybir.ActivationFunctionType
    ALU = mybir.AluOpType

    nc = bacc.Bacc("TRN2", target_bir_lowering=False, debug=False,
                   num_devices=NCORES, enable_partition_id=False)

    xb_d = nc.dram_tensor("xb", [TPAD, D], bf16, kind="ExternalInput")
    xq_d = nc.dram_tensor("xq", [QPAD, D], bf16, kind="ExternalInput")
    # [c(128), set(2:q,k), j(3), cb(2), db(2), d(128)]
    wqk_d = nc.dram_tensor("wqk", [128, 2, KW, 2, 2, 128], bf16,
                           kind="ExternalInput")
    # [c(128), j(3), cb(2), col(264)]
    wv_d = nc.dram_tensor("wv", [128, KW, 2, H * CA], bf16, kind="ExternalInput")
    # [d(128), set(2), db(2)]
    bqk_d = nc.dram_tensor("bqk", [128, 2, 2], f32, kind="ExternalInput")
    vb_d = nc.dram_tensor("vbias", [H * CA], f32, kind="ExternalInput")
    # [dd(32), h(8), o(256)]
    outw_d = nc.dram_tensor("outw", [DK, H, D], bf16, kind="ExternalInput")
    outb_d = nc.dram_tensor("outb", [D], f32, kind="ExternalInput")
    out_d = nc.dram_tensor("out", [QS, D], f32, kind="ExternalOutput")
    dbg = {}
    if debug_dump:
        dbg["kt"] = nc.dram_tensor("dbg_kt", [128, 2, 2, T], bf16,
                                   kind="ExternalOutput")
        dbg["qt"] = nc.dram_tensor("dbg_qt", [128, 2, 2, QS], bf16,
                                   kind="ExternalOutput")
        dbg["vt"] = nc.dram_tensor("dbg_vt", [128, NKB, H * CA], bf16,
                                   kind="ExternalOutput")
        dbg["ct"] = nc.dram_tensor("dbg_ct", [2, DK, H, QB], bf16,
                                   kind="ExternalOutput")
        dbg["pt"] = nc.dram_tensor("dbg_pt", [128, 3, QB], bf16,
                                   kind="ExternalOutput")
        dbg["nrm"] = nc.dram_tensor("dbg_nrm", [4, CA, QB], f32,
                                    kind="ExternalOutput")

    with tile.TileContext(nc) as tc:
        with (
            tc.tile_pool(name="consts", bufs=1) as consts,
            tc.tile_pool(name="work", bufs=3) as work,
            tc.tile_pool(name="psum", bufs=2, space=bass.MemorySpace.PSUM) as psum,
        ):
            # ---- constants / weights ----
            wqk_sb = consts.tile([128, 2, KW, 2, 2, 128], bf16)
            nc.sync.dma_start(wqk_sb[:], wqk_d[:])
            wv_sb = consts.tile([128, KW, 2, H * CA], bf16)
            nc.sync.dma_start(wv_sb[:], wv_d[:])
            bqk_sb = consts.tile([128, 2, 2], f32)
            nc.sync.dma_start(bqk_sb[:], bqk_d[:])
            outw_sb = consts.tile([DK, H, D], bf16)
            nc.sync.dma_start(outw_sb[:], outw_d[:])
            # partition-broadcast bias rows
            vb_sb = consts.tile([128, H * CA], f32)
            vb_ap = bass.AP(tensor=vb_d, offset=0, ap=[[0, 128], [1, H * CA]])
            nc.sync.dma_start(vb_sb[:], vb_ap)
            outb_sb = consts.tile([128, D], f32)
            outb_ap = bass.AP(tensor=outb_d, offset=0, ap=[[0, 128], [1, D]])
            nc.sync.dma_start(outb_sb[:], outb_ap)
            ones_sb = consts.tile([CA, DK], f32)
            nc.vector.memset(ones_sb[:], 1.0)

            # ---- x^T loads (DMA transpose, bf16) ----
            xbT = consts.tile([128, 2, TPAD], bf16)
            xqT = consts.tile([128, 2, QPAD], bf16)
            for cb in range(2):
                nc.sync.dma_start_transpose(
                    xbT[:, cb, :], xb_d[:, cb * 128:(cb + 1) * 128])
                nc.sync.dma_start_transpose(
                    xqT[:, cb, :], xq_d[:, cb * 128:(cb + 1) * 128])

            # alt-strip replicated K^T/Q^T: [part(strip), hq, alt, t];
            # alt=1 holds the same data partition-rotated by 64 so consecutive
            # key blocks' score matmuls hit different PE row groups
            # (LDWEIGHTS of kb+1 overlaps the matmul of kb).
            KT = consts.tile([128, 2, 2, T], bf16)
            QT = consts.tile([128, 2, 2, QS], bf16)
            Vt = consts.tile([128, NKB, H * CA], bf16)

            def proj_qk():
                for dst, src, n_t, s in ((KT, xbT, T, 1), (QT, xqT, QS, 0)):
                    for db in range(2):
                        for tt in range(n_t // QB):
                            ps = psum.tile([128, 3, QB], f32, tag="scores",
                                           name="ps_proj")
                            first = True
                            for j in range(KW):
                                for cb in range(2):
                                    nc.tensor.matmul(
                                        ps[:, 0, :],
                                        lhsT=wqk_sb[:, s, j, cb, db, :],
                                        rhs=src[:, cb,
                                                tt * QB + j: tt * QB + j + QB],
                                        start=first,
                                        stop=(j == KW - 1 and cb == 1))
                                    first = False
                            sl = slice(tt * QB, (tt + 1) * QB)
                            nc.vector.tensor_scalar_add(
                                dst[:, db, 0, sl], ps[:, 0, :],
                                bqk_sb[:, s, db:db + 1])
                            # partition-rotated replica for strip alternation
                            nc.sync.dma_start(dst[64:128, db, 1, sl],
                                              dst[0:64, db, 0, sl])
                            nc.sync.dma_start(dst[0:64, db, 1, sl],
                                              dst[64:128, db, 0, sl])

            def proj_v():
                for kb in range(NKB):
                    ps = psum.tile([128, 3, QB], f32, tag="scores", name="ps_v")
                    first = True
                    for j in range(KW):
                        for cb in range(2):
                            nc.tensor.matmul(
                                ps[:, 0, :H * CA],
                                lhsT=xbT[:, cb, kb * 128 + j: kb * 128 + j + 128],
                                rhs=wv_sb[:, j, cb, :],
                                start=first, stop=(j == KW - 1 and cb == 1))
                            first = False
                    nc.vector.tensor_add(Vt[:, kb, :], ps[:, 0, :H * CA],
                                         vb_sb[:])

            # key-block groups of 3 (psum: 2x3 score banks + 2x1 ctx banks = 8)
            groups = [(k, min(k + 3, NKB)) for k in range(0, NKB, 3)]

            def scores_exp(qb, h, k0, k1, pt_tag="PT", pt_bufs=None):
                """Emit score matmuls + exp for one key-block group; returns PT."""
                hp, hq = h % 4, h // 4
                nkb = k1 - k0
                sp = psum.tile([128, 3, QB], f32, tag="scores", name="sp")
                for i in range(nkb):
                    kb = k0 + i
                    alt = kb & 1
                    st32 = (hp ^ (2 * alt)) * 32
                    nc.tensor.matmul(
                        sp[:, i, :],
                        lhsT=KT[st32:st32 + 32, hq, alt,
                                kb * 128:(kb + 1) * 128],
                        rhs=QT[st32:st32 + 32, hq, alt,
                               qb * QB:(qb + 1) * QB],
                        start=True, stop=True,
                        tile_position=(st32, 0))
                PT = work.tile([128, 3, QB], bf16, tag=pt_tag, bufs=pt_bufs,
                               name="PT")
                nc.scalar.activation(PT[:, :nkb, :], sp[:, :nkb, :], AF.Exp)
                return PT

            def av_group(ctxp, h, k0, k1, PT):
                for i in range(k1 - k0):
                    kb = k0 + i
                    nc.tensor.matmul(
                        ctxp[:],
                        lhsT=Vt[:, kb, h * CA:(h + 1) * CA],
                        rhs=PT[:, i, :],
                        start=(kb == 0), stop=(kb == NKB - 1))

            def normalize(CT, qb, h, ctxp, dump=False):
                # CT[:, h, :] = ctx_unnorm * bcast(1/denom)
                den_sb = work.tile([CA, QB], f32, tag="den")
                nc.vector.tensor_copy(den_sb[DK:CA, :], ctxp[DK:CA, :])
                rec_sb = work.tile([CA, QB], f32, tag="rec")
                nc.vector.reciprocal(rec_sb[DK:CA, :], den_sb[DK:CA, :])
                bc = psum.tile([DK, QB], f32, tag="scores", name="bc")
                nc.tensor.matmul(bc[:], lhsT=ones_sb[DK:CA, :],
                                 rhs=rec_sb[DK:CA, :], start=True, stop=True,
                                 tile_position=(32, 0))
                bc_sb = work.tile([DK, QB], f32, tag="bcsb")
                nc.vector.tensor_copy(bc_sb[:], bc[:])
                if dump:
                    ctxu_sb = work.tile([CA, QB], f32, tag="dbgc")
                    nc.vector.tensor_copy(ctxu_sb[:], ctxp[:])
                    nc.sync.dma_start(dbg["nrm"][0], ctxu_sb[:])
                    nc.sync.dma_start(dbg["nrm"][1, :, :], den_sb[:])
                    nc.sync.dma_start(dbg["nrm"][2, :, :], rec_sb[:])
                    nc.sync.dma_start(dbg["nrm"][3, :DK, :], bc_sb[:])
                nc.vector.tensor_tensor(
                    CT[:, h, :], ctxp[:DK, :], bc_sb[:], ALU.mult)

            def attn_head(CT, qb, h, dump=False):
                ctxp = psum.tile([CA, QB], f32, tag="ctx", name="ctxp")
                for (k0, k1) in groups:
                    PT = scores_exp(qb, h, k0, k1)
                    if dump and qb == 0 and h == 0 and k0 == 0:
                        nc.sync.dma_start(dbg["pt"][:], PT[:])
                    av_group(ctxp, h, k0, k1, PT)
                normalize(CT, qb, h, ctxp, dump=dump and qb == 0 and h == 0)

            def outproj(CT, qb):
                for qs in range(QB // 128):
                    op = psum.tile([128, 3, QB], f32, tag="scores", name="op")
                    for h in range(H):
                        nc.tensor.matmul(
                            op[:, 0, :D],
                            lhsT=CT[:, h, qs * 128:(qs + 1) * 128],
                            rhs=outw_sb[:, h, :],
                            start=(h == 0), stop=(h == H - 1))
                    osb = work.tile([128, D], f32, tag="osb")
                    nc.vector.tensor_add(osb[:], op[:, 0, :D], outb_sb[:])
                    nc.sync.dma_start(
                        out_d[qb * QB + qs * 128: qb * QB + (qs + 1) * 128, :],
                        osb[:])

            for rep in range(reps):
                proj_qk()
                # pre-emit head-0 scores+exp so ACT works during V projection;
                # the 11 PT tiles stay live until their deferred AV matmuls.
                CT0 = work.tile([DK, H, QB], bf16, tag="CT", bufs=2, name="CT")
                pts0 = [(k0, k1, scores_exp(0, 0, k0, k1,
                                            pt_tag="PT0", pt_bufs=len(groups)))
                        for (k0, k1) in groups]
                proj_v()
                if debug_dump and rep == 0:
                    nc.sync.dma_start(dbg["kt"][:], KT[:])
                    nc.sync.dma_start(dbg["qt"][:], QT[:])
                    nc.sync.dma_start(dbg["vt"][:], Vt[:])
                ctxp0 = psum.tile([CA, QB], f32, tag="ctx", name="ctxp")
                for (k0, k1, PT) in pts0:
                    av_group(ctxp0, 0, k0, k1, PT)
                normalize(CT0, 0, 0, ctxp0, dump=debug_dump and rep == 0)
                for h in range(1, H):
                    attn_head(CT0, 0, h, dump=debug_dump and rep == 0)
                outproj(CT0, 0)
                CT1 = work.tile([DK, H, QB], bf16, tag="CT", bufs=2, name="CT")
                for h in range(H):
                    attn_head(CT1, 1, h, dump=False)
                outproj(CT1, 1)

    nc.compile()
    return nc


def _prep_inputs(x, dw_w, dw_b, bn_gamma, bn_beta, bn_mean, bn_var,
                 pw_w, pw_b, out_w, out_b):
    """Host-side arrangement of per-core input dicts."""
    bf = ml_dtypes.bfloat16
    Wj, beta = _fold_weights(dw_w, dw_b, bn_gamma, bn_beta, bn_mean,
                             bn_var, pw_w, pw_b)

    # wqk [c, set, j, cb, db, d] = Wj[set, j, db*128+d, cb*128+c]
    w2 = Wj[:2].reshape(2, KW, 2, 128, 2, 128)  # [set, j, db, d, cb, c]
    wqk = np.ascontiguousarray(w2.transpose(5, 0, 1, 4, 2, 3)).astype(bf)

    # wv [c, j, cb, col] with col = h*33+dd (dd<32), ones col zero-weight
    wv = np.zeros((128, KW, 2, H * CA), dtype=np.float32)
    wv3 = Wj[2].reshape(KW, H, DK, 2, 128)  # [j, h, dd, cb, c]
    for h in range(H):
        wv[:, :, :, h * CA:h * CA + DK] = wv3[:, h].transpose(3, 0, 2, 1)
    wv = wv.astype(bf)

    bqk = np.ascontiguousarray(
        beta[:2].reshape(2, 2, 128).transpose(2, 0, 1)).astype(np.float32)

    vbias = np.zeros((H * CA,), dtype=np.float32)
    vb3 = beta[2].reshape(H, DK)
    for h in range(H):
        vbias[h * CA:h * CA + DK] = vb3[h]
        vbias[h * CA + DK] = 1.0

    # outw [dd, h, o] = out_w[o, h*32+dd]
    outw = np.ascontiguousarray(
        out_w.reshape(D, H, DK).transpose(2, 1, 0)).astype(bf)
    outb = out_b.astype(np.float32)

    shared = dict(wqk=wqk, wv=wv, bqk=bqk, vbias=vbias, outw=outw, outb=outb)

    xpad = np.zeros((B, TPAD, D), dtype=bf)
    xpad[:, 1:T + 1, :] = x.astype(bf)

    in_maps = []
    for core in range(NCORES):
        b, q0 = core // 4, (core % 4) * QS
        m = dict(shared)
        m["xb"] = np.ascontiguousarray(xpad[b])
        m["xq"] = np.ascontiguousarray(xpad[b, q0:q0 + QPAD])
        in_maps.append(m)
    return in_maps


LAST_RESULTS = None


def _get_exec(reps=1):
    """Build the bass module once and wrap it in a cached, jitted 8-core
    shard_map callable (PJRT / axon path)."""
    key = ("exec", reps)
    if key in _cached:
        return _cached[key]
    import jax
    from jax.sharding import Mesh, PartitionSpec
    from jax.experimental.shard_map import shard_map
    import concourse.mybir as mybir
    from concourse import bass2jax

    bass2jax.install_neuronx_cc_hook()
    nc = _build_nc(reps=reps)

    in_names, out_names, out_avals = [], [], []
    for alloc in nc.m.functions[0].allocations:
        if not isinstance(alloc, mybir.MemoryLocationSet):
            continue
        name = alloc.memorylocations[0].name
        if alloc.kind == "ExternalInput":
            in_names.append(name)
        elif alloc.kind == "ExternalOutput":
            out_names.append(name)
            out_avals.append(jax.core.ShapedArray(
                tuple(alloc.tensor_shape), mybir.dt.np(alloc.dtype)))
    all_in_names = in_names + out_names  # outputs passed as zero inputs

    def _body(*args):
        outs = bass2jax._bass_exec_p.bind(
            *args,
            out_avals=tuple(out_avals),
            in_names=tuple(all_in_names),
            out_names=tuple(out_names),
            lowering_input_output_aliases=(),
            sim_require_finite=True,
            sim_require_nnan=True,
            nc=nc,
        )
        return tuple(outs)

    devices = jax.devices()[:NCORES]
    mesh = Mesh(np.asarray(devices), ("core",))
    n_all = len(in_names) + len(out_names)
    fn = jax.jit(
        shard_map(_body, mesh=mesh,
                  in_specs=(PartitionSpec("core"),) * n_all,
                  out_specs=(PartitionSpec("core"),) * len(out_names),
                  check_rep=False),
        keep_unused=True,
    )
    _cached[key] = (fn, in_names, out_names, out_avals, mesh)
    return _cached[key]


def _device_args(in_maps, reps=1):
    """Concat per-core inputs on axis 0 and device_put with core sharding."""
    import jax
    from jax.sharding import NamedSharding, PartitionSpec
    fn, in_names, out_names, out_avals, mesh = _get_exec(reps)
    sh = NamedSharding(mesh, PartitionSpec("core"))
    args = []
    for name in in_names:
        cat = np.concatenate([in_maps[c][name][None] for c in range(NCORES)],
                             axis=0)
        cat = cat.reshape(NCORES * cat.shape[1], *cat.shape[2:])
        args.append(jax.device_put(cat, sh))
    for av in out_avals:
        z = np.zeros((NCORES * av.shape[0], *av.shape[1:]), av.dtype)
        args.append(jax.device_put(z, sh))
    return args


def _run(args, reps=1):
    fn, in_names, out_names, out_avals, mesh = _get_exec(reps)
    outs = fn(*args)
    res = []
    for c in range(NCORES):
        res.append({name: np.asarray(outs[i]).reshape(
            NCORES, *out_avals[i].shape)[c] for i, name in enumerate(out_names)})
    return res, outs


def kernel(x, dw_w, dw_b, bn_gamma, bn_beta, bn_mean, bn_var,
           pw_w, pw_b, out_w, out_b):
    global LAST_RESULTS
    args = [np.asarray(a) for a in (x, dw_w, dw_b, bn_gamma, bn_beta, bn_mean,
                                    bn_var, pw_w, pw_b, out_w, out_b)]
    with _lock:
        in_maps = _prep_inputs(*args)
        dev_args = _device_args(in_maps)
        _cached["bench_args"] = dev_args
        results, _ = _run(dev_args)
    LAST_RESULTS = results

    out = np.empty((B, T, D), dtype=np.float32)
    for core in range(NCORES):
        b, q0 = core // 4, (core % 4) * QS
        out[b, q0:q0 + QS] = results[core]["out"]
    return out


def bench(n=6, reps=1):
    """Steady-state wall time of the jitted 8-core execution."""
    import time
    import jax
    fn, in_names, out_names, out_avals, mesh = _get_exec(reps)
    dev_args = _cached["bench_args"]
    # warmup for this reps variant
    outs = fn(*dev_args)
    jax.block_until_ready(outs)
    times = []
    for _ in range(n):
        t0 = time.perf_counter()
        outs = fn(*dev_args)
        jax.block_until_ready(outs)
        times.append(time.perf_counter() - t0)
    return times
